# revision 23
# baseline (speedup 1.0000x reference)
"""Trainium2 Bass kernel for the MHC layer (nn_MHCLayer_20555713478899).

Reference computation (per batch row b of x[B=8192, n=4, C=4096] f32):
    hpre = sigmoid(H_pre)                     # [4]
    x_agg[b, c]   = sum_n hpre[n] * x[b, n, c]
    x_agg_bf      = bf16_roundtrip(x_agg)
    rms[b]        = sqrt(mean_c(x_agg_bf^2) + 1e-6)
    y_norm[b, c]  = x_agg_bf / rms * rmsnorm_weight[c]
    P             = sinkhorn3(exp(H_res))     # [4, 4]  (tiny, host-computed)
    hpost = 2*sigmoid(H_post)                 # [4]
    out[b, i, c]  = sum_j P[i, j] * x[b, j, c] + hpost[i] * y_norm[b, c]

Strategy: data-parallel shard of B across 8 NeuronCores (1024 rows each).
On-chip, batches are processed in supertiles of 128 rows = 4 subtiles of 32
rows.  A supertile loads as ONE 4MB DMA into a [128, 4*4096] SBUF tile whose
partition index is (bg*4 + n) and whose free index is (s*4096 + c); the
n-mixing then runs as 128-partition matmuls with small block matrices:
  agg : 4 concurrent col-tiles (M=32, tile_position=(0,32s)) write
        x_agg rows 32s+bg of one [128,512] PSUM chunk
  mix : full-array blockP (block-diagonal P.T), one mm per subtile chunk
  post: row-tiles (K=32, tile_position=(32s,0)) accumulate
        hpost[i]*y_norm into the mix PSUM, 2 subtiles concurrently
The RMS-norm path: ACT squares the agg PSUM directly (accum_out), DVE
evacuates agg PSUM fused with the rmsnorm weight multiply, Rsqrt gives
1/rms, and y_norm = xagg_w * invr on DVE.  Mix PSUM is evacuated to bf16
by alternating DVE/ACT copies.

HBM traffic is halved vs the f32 version: x is cast to bf16 on the host
(the device consumed bf16 for every matmul anyway) and the output is
stored as bf16 and upcast to f32 on the host.  Loads ride HWDGE (sync/SP
ring); stores ride SWDGE (gpsimd/Pool ring) so the two streams sit on
different DMA queues and interleave at packet granularity.
"""

import contextlib
import os

import numpy as np
import ml_dtypes

import concourse.bass as bass
import concourse.tile as tile
from concourse import bacc, mybir
from concourse.bass_utils import run_bass_kernel_spmd

B, N, C = 8192, 4, 4096
NCORES = 8
BLOC = B // NCORES          # 1024 batch rows per core
SUB = 32                    # batch rows per subtile (SUB*N = 128 partitions)
NSUB = 4                    # subtiles per supertile
ST = SUB * NSUB             # 128 batch rows per supertile
CH = 512                    # matmul / PSUM chunk width (one PSUM bank)
NCH = C // CH               # 8 chunks per subtile row
EPS = 1e-6
SINKHORN_ITERS = 3

F32 = mybir.dt.float32
BF16 = mybir.dt.bfloat16
BF16_NP = ml_dtypes.bfloat16

_PROGRAM = None
LAST_RESULTS = None         # BassKernelResults of the last run (for profiling)


def _build_program(bloc=BLOC):
    nc = bacc.Bacc("TRN2", target_bir_lowering=False)

    x_d = nc.dram_tensor("x", [bloc, N, C], BF16, kind="ExternalInput")
    wrep_d = nc.dram_tensor("wrep", [128, C], BF16, kind="ExternalInput")
    blockp_d = nc.dram_tensor("blockp", [128, 128], BF16, kind="ExternalInput")
    wpre_d = nc.dram_tensor("wpre", [128, SUB], BF16, kind="ExternalInput")
    bpost_d = nc.dram_tensor("bpost", [128, 128], BF16, kind="ExternalInput")
    out_d = nc.dram_tensor("out", [bloc, N, C], BF16, kind="ExternalOutput")

    n_st = bloc // ST
    AluOp = mybir.AluOpType
    Act = mybir.ActivationFunctionType

    # Half-supertile views: 64 rows = 2 subtiles; partition (bg n), free
    # (s c).  One 2MB DMA per half for finer load/store pipelining.
    xv = x_d[:].rearrange("(th s bg) n c -> th (bg n) s c", s=2, bg=SUB)
    ov = out_d[:].rearrange("(th s bg) n c -> th (bg n) s c", s=2, bg=SUB)

    with tile.TileContext(nc) as tc:
        with (
            tc.tile_pool(name="consts", bufs=1) as consts,
            tc.tile_pool(name="xsup", bufs=6) as x_pool,
            tc.tile_pool(name="aggw", bufs=2) as aggw_pool,
            tc.tile_pool(name="yn", bufs=2) as yn_pool,
            tc.tile_pool(name="scr", bufs=2) as scr_pool,
            tc.tile_pool(name="small", bufs=4) as small_pool,
            tc.tile_pool(name="osb", bufs=4) as out_pool,
            tc.tile_pool(name="aggps", bufs=4, space=bass.MemorySpace.PSUM) as agg_pool,
            tc.tile_pool(name="mixps", bufs=2, space=bass.MemorySpace.PSUM) as mix_pool,
        ):
            wrep_t = consts.tile([128, C], BF16, tag="wrep", name="wrep_t")
            nc.sync.dma_start(wrep_t[:], wrep_d[:])
            blockp_t = consts.tile([128, 128], BF16, tag="blockp", name="blockp_t")
            nc.sync.dma_start(blockp_t[:], blockp_d[:])
            wpre_t = consts.tile([128, SUB], BF16, tag="wpre", name="wpre_t")
            nc.sync.dma_start(wpre_t[:], wpre_d[:])
            bpost_t = consts.tile([128, 128], BF16, tag="bpost", name="bpost_t")
            nc.sync.dma_start(bpost_t[:], bpost_d[:])
            eps_t = consts.tile([128, 1], F32, tag="eps", name="eps_t")
            nc.vector.memset(eps_t[:], EPS)

            state = {}

            def emit_load(th):
                xt = x_pool.tile([128, 2, C], BF16, tag="xsup",
                                 name=f"x_{th}")
                nc.sync.dma_start(out=xt[:], in_=xv[th])
                t, h = divmod(th, 2)
                if t not in state:
                    state[t] = [[None, None], None, None]
                state[t][0][h] = xt

            def emit_agg(t):
                """x_agg via 4 concurrent col-tiled matmuls per 512-chunk.

                PSUM partition 32s+bg holds x_agg of supertile row 32s+bg.
                DVE evacuates fused with the rmsnorm-weight multiply; ACT
                squares the (unrounded) PSUM values with accum_out.
                """
                xh = state[t][0]
                xaggw = aggw_pool.tile([128, C], BF16, tag="xaggw",
                                       name=f"xaggw_{t}")
                sq8 = small_pool.tile([128, NCH], F32, tag="sq8",
                                      name=f"sq8_{t}")
                ats = {}
                # Emit the half-A col-tiles for all chunks first: they only
                # depend on the first 2MB load, so the PE can start while
                # half B is still in flight (PE queue is FIFO).
                for q in range(NCH):
                    at = agg_pool.tile([128, CH], F32, tag="agg",
                                       name=f"agg_{t}_{q}")
                    ats[q] = at
                    for s in (0, 1):
                        nc.tensor.matmul(
                            at[32 * s : 32 * s + 32, :],
                            wpre_t[:],
                            xh[0][:, s, CH * q : CH * (q + 1)],
                            start=True,
                            stop=True,
                            tile_position=(0, 32 * s),
                        )
                    if q == NCH // 2 - 1:
                        break
                for q in range(NCH):
                    if q < NCH // 2:
                        at = ats[q]
                    else:
                        at = agg_pool.tile([128, CH], F32, tag="agg",
                                           name=f"agg_{t}_{q}")
                        for s in (0, 1):
                            nc.tensor.matmul(
                                at[32 * s : 32 * s + 32, :],
                                wpre_t[:],
                                xh[0][:, s, CH * q : CH * (q + 1)],
                                start=True,
                                stop=True,
                                tile_position=(0, 32 * s),
                            )
                    for s in (2, 3):
                        nc.tensor.matmul(
                            at[32 * s : 32 * s + 32, :],
                            wpre_t[:],
                            xh[1][:, s - 2, CH * q : CH * (q + 1)],
                            start=True,
                            stop=True,
                            tile_position=(0, 32 * s),
                        )
                    lo = CH * q
                    nc.vector.tensor_mul(
                        xaggw[:, lo : lo + CH], at[:], wrep_t[:, lo : lo + CH]
                    )
                    scr = scr_pool.tile([128, CH], BF16, tag="scr",
                                        name=f"scr_{t}_{q}")
                    nc.scalar.activation(
                        scr[:], at[:], Act.Square, accum_out=sq8[:, q : q + 1]
                    )
                state[t][1] = (xaggw, sq8)

            def emit_norm(t):
                xaggw, sq8 = state[t][1]
                sumsq = small_pool.tile([128, 1], F32, tag="sumsq",
                                        name=f"ss_{t}")
                nc.vector.tensor_reduce(
                    sumsq[:], sq8[:], mybir.AxisListType.X, AluOp.add
                )
                rmsv = small_pool.tile([128, 1], F32, tag="rmsv",
                                       name=f"rms_{t}")
                nc.scalar.activation(
                    rmsv[:], sumsq[:], Act.Sqrt, bias=eps_t[:], scale=1.0 / C
                )
                invr = small_pool.tile([128, 1], F32, tag="invr",
                                       name=f"invr_{t}")
                nc.vector.reciprocal(invr[:], rmsv[:])
                yn = yn_pool.tile([128, C], BF16, tag="yn", name=f"yn_{t}")
                nc.vector.tensor_scalar_mul(yn[:], xaggw[:], invr[:])
                state[t][2] = yn

            def emit_mix(t):
                """mix + post per 512-chunk, two subtiles at a time.

                blockp is a full-array matmul; the two posts are K=32
                row-tiles at tile_position (32s, 0) and run concurrently.
                Evacuation alternates DVE / ACT.
                """
                xh, _, yn = state.pop(t)
                evac = 0
                for pair in ((0, 1), (2, 3)):
                    h = pair[0] // 2
                    osb = out_pool.tile([128, 2, C], BF16, tag="osb",
                                        name=f"osb_{t}_{h}")
                    for q in range(NCH):
                        mts = []
                        for s in pair:
                            mt = mix_pool.tile([128, CH], F32, tag=f"mix{s % 2}",
                                           name=f"mix_{t}_{s}_{q}")
                            nc.tensor.matmul(
                                mt[:],
                                blockp_t[:],
                                xh[s // 2][:, s % 2, CH * q : CH * (q + 1)],
                                start=True,
                                stop=False,
                            )
                            mts.append((s, mt))
                        for s, mt in mts:
                            nc.tensor.matmul(
                                mt[:],
                                bpost_t[32 * s : 32 * s + 32, :],
                                yn[32 * s : 32 * s + 32, CH * q : CH * (q + 1)],
                                start=False,
                                stop=True,
                                tile_position=(32 * s, 0),
                            )
                        for s, mt in mts:
                            dst = osb[:, s % 2, CH * q : CH * (q + 1)]
                            if evac % 2 == 0:
                                nc.vector.tensor_copy(dst, mt[:])
                            else:
                                nc.scalar.copy(dst, mt[:])
                            evac += 1
                    nc.gpsimd.dma_start(out=ov[2 * t + h], in_=osb[:])

            n_th = 2 * n_st
            for th in range(min(4, n_th)):
                emit_load(th)
            for t in range(n_st):
                for th in (2 * t + 4, 2 * t + 5):
                    if th < n_th:
                        emit_load(th)
                if t == 0:
                    emit_agg(0)
                emit_norm(t)
                if t + 1 < n_st:
                    emit_agg(t + 1)
                emit_mix(t)

    nc.compile()
    return nc


def _sigmoid_f32(x):
    x = np.asarray(x, np.float32)
    return (1.0 / (1.0 + np.exp(-x.astype(np.float64)))).astype(np.float32)


def _host_matrices(rmsnorm_weight, H_pre, H_post, H_res):
    f32 = np.float32
    hpre = _sigmoid_f32(H_pre)                        # [4]
    hpost = (2.0 * _sigmoid_f32(H_post)).astype(f32)  # [4]
    P = np.exp(np.asarray(H_res, f32))
    for _ in range(SINKHORN_ITERS):
        P = P / (P.sum(axis=-1, keepdims=True) + f32(EPS))
        P = P / (P.sum(axis=-2, keepdims=True) + f32(EPS))
    P = P.astype(f32)

    # mix: out[(bg,i), c] = sum_j blockp[(bg,j), (bg,i)] * x[(bg,j), c]
    blockp = np.zeros((128, 128), f32)
    for bg in range(SUB):
        blockp[4 * bg : 4 * bg + 4, 4 * bg : 4 * bg + 4] = P.T

    # agg col-tile weights (same for every col group s):
    #   out[col 32s+bg2, c] = sum_(bg,n) wpre[(bg,n), bg2] * x_s[(bg,n), c]
    wpre = np.zeros((128, SUB), f32)
    for bg in range(SUB):
        for n in range(4):
            wpre[4 * bg + n, bg] = hpre[n]

    # post row-tile weights: rows 32s.. hold the K=32 lhsT for subtile s:
    #   out[(bg,i), c] += sum_bg2 bpost[32s+bg2, (bg,i)] * yn[32s+bg2, c]
    bpost = np.zeros((128, 128), f32)
    for s in range(NSUB):
        for bg in range(SUB):
            for i in range(4):
                bpost[32 * s + bg, 4 * bg + i] = hpost[i]

    wrep = np.broadcast_to(
        np.asarray(rmsnorm_weight, f32)[None, :], (128, C)
    )
    return {
        "wrep": np.ascontiguousarray(wrep.astype(BF16_NP)),
        "blockp": blockp.astype(BF16_NP),
        "wpre": wpre.astype(BF16_NP),
        "bpost": bpost.astype(BF16_NP),
    }


def kernel(x, rmsnorm_weight, H_pre, H_post, H_res):
    global _PROGRAM, LAST_RESULTS
    x = np.asarray(x, np.float32)
    assert x.shape == (B, N, C), x.shape
    xbf = np.ascontiguousarray(x.astype(BF16_NP))

    if _PROGRAM is None:
        _PROGRAM = _build_program()
    nc = _PROGRAM

    consts = _host_matrices(rmsnorm_weight, H_pre, H_post, H_res)
    shards = np.split(xbf, NCORES, axis=0)
    in_maps = [{"x": np.ascontiguousarray(s), **consts} for s in shards]

    trace = bool(int(os.environ.get("MHC_TRACE", "0")))
    br = run_bass_kernel_spmd(
        nc, in_maps, core_ids=list(range(NCORES)), trace=trace
    )
    LAST_RESULTS = br
    out = np.concatenate([r["out"] for r in br.results], axis=0)
    return out.astype(np.float32)


# revision 24
# speedup vs baseline: 1.0439x; 1.0439x over previous
"""Trainium2 Bass kernel for the MHC layer (nn_MHCLayer_20555713478899).

Reference computation (per batch row b of x[B=8192, n=4, C=4096] f32):
    hpre = sigmoid(H_pre)                     # [4]
    x_agg[b, c]   = sum_n hpre[n] * x[b, n, c]
    x_agg_bf      = bf16_roundtrip(x_agg)
    rms[b]        = sqrt(mean_c(x_agg_bf^2) + 1e-6)
    y_norm[b, c]  = x_agg_bf / rms * rmsnorm_weight[c]
    P             = sinkhorn3(exp(H_res))     # [4, 4]  (tiny, host-computed)
    hpost = 2*sigmoid(H_post)                 # [4]
    out[b, i, c]  = sum_j P[i, j] * x[b, j, c] + hpost[i] * y_norm[b, c]

Strategy: data-parallel shard of B across 8 NeuronCores (1024 rows each).
On-chip, batches are processed in supertiles of 128 rows = 4 subtiles of 32
rows.  A supertile loads as ONE 4MB DMA into a [128, 4*4096] SBUF tile whose
partition index is (bg*4 + n) and whose free index is (s*4096 + c); the
n-mixing then runs as 128-partition matmuls with small block matrices:
  agg : 4 concurrent col-tiles (M=32, tile_position=(0,32s)) write
        x_agg rows 32s+bg of one [128,512] PSUM chunk
  mix : full-array blockP (block-diagonal P.T), one mm per subtile chunk
  post: row-tiles (K=32, tile_position=(32s,0)) accumulate
        hpost[i]*y_norm into the mix PSUM, 2 subtiles concurrently
The RMS-norm path: ACT squares the agg PSUM directly (accum_out), DVE
evacuates agg PSUM fused with the rmsnorm weight multiply, Rsqrt gives
1/rms, and y_norm = xagg_w * invr on DVE.  Mix PSUM is evacuated to bf16
by alternating DVE/ACT copies.

HBM traffic is halved vs the f32 version: x is cast to bf16 on the host
(the device consumed bf16 for every matmul anyway) and the output is
stored as bf16 and upcast to f32 on the host.  Loads ride HWDGE (sync/SP
ring); stores ride SWDGE (gpsimd/Pool ring) so the two streams sit on
different DMA queues and interleave at packet granularity.
"""

import contextlib
import os

import numpy as np
import ml_dtypes

import concourse.bass as bass
import concourse.tile as tile
from concourse import bacc, mybir
from concourse.bass_utils import run_bass_kernel_spmd

B, N, C = 8192, 4, 4096
NCORES = 8
BLOC = B // NCORES          # 1024 batch rows per core
SUB = 32                    # batch rows per subtile (SUB*N = 128 partitions)
NSUB = 4                    # subtiles per supertile
ST = SUB * NSUB             # 128 batch rows per supertile
CH = 512                    # matmul / PSUM chunk width (one PSUM bank)
NCH = C // CH               # 8 chunks per subtile row
EPS = 1e-6
SINKHORN_ITERS = 3

F32 = mybir.dt.float32
BF16 = mybir.dt.bfloat16
BF16_NP = ml_dtypes.bfloat16

_PROGRAM = None
LAST_RESULTS = None         # BassKernelResults of the last run (for profiling)


def _build_program(bloc=BLOC):
    nc = bacc.Bacc("TRN2", target_bir_lowering=False)

    x_d = nc.dram_tensor("x", [bloc, N, C], BF16, kind="ExternalInput")
    wrep_d = nc.dram_tensor("wrep", [128, C], BF16, kind="ExternalInput")
    blockp_d = nc.dram_tensor("blockp", [128, 128], BF16, kind="ExternalInput")
    wpre_d = nc.dram_tensor("wpre", [128, SUB], BF16, kind="ExternalInput")
    bpost_d = nc.dram_tensor("bpost", [128, 128], BF16, kind="ExternalInput")
    out_d = nc.dram_tensor("out", [bloc, N, C], BF16, kind="ExternalOutput")

    n_st = bloc // ST
    AluOp = mybir.AluOpType
    Act = mybir.ActivationFunctionType

    # Half-supertile views: 64 rows = 2 subtiles; partition (bg n), free
    # (s c).  One 2MB DMA per half for finer load/store pipelining.
    xv = x_d[:].rearrange("(th s bg) n c -> th (bg n) s c", s=2, bg=SUB)
    ov = out_d[:].rearrange("(th s bg) n c -> th (bg n) s c", s=2, bg=SUB)

    with tile.TileContext(nc) as tc:
        with (
            tc.tile_pool(name="consts", bufs=1) as consts,
            tc.tile_pool(name="xsup", bufs=6) as x_pool,
            tc.tile_pool(name="aggw", bufs=2) as aggw_pool,
            tc.tile_pool(name="yn", bufs=2) as yn_pool,
            tc.tile_pool(name="scr", bufs=2) as scr_pool,
            tc.tile_pool(name="small", bufs=4) as small_pool,
            tc.tile_pool(name="osb", bufs=4) as out_pool,
            tc.tile_pool(name="aggps", bufs=4, space=bass.MemorySpace.PSUM) as agg_pool,
            tc.tile_pool(name="mixps", bufs=2, space=bass.MemorySpace.PSUM) as mix_pool,
        ):
            # wpre (8KB) is all the first agg matmuls need; the larger
            # consts are deferred until after the first two x loads so the
            # PE can start ~10us earlier.
            wpre_t = consts.tile([128, SUB], BF16, tag="wpre", name="wpre_t")
            nc.sync.dma_start(wpre_t[:], wpre_d[:])
            eps_t = consts.tile([128, 1], F32, tag="eps", name="eps_t")
            nc.vector.memset(eps_t[:], EPS)
            wrep_t = consts.tile([128, C], BF16, tag="wrep", name="wrep_t")
            blockp_t = consts.tile([128, 128], BF16, tag="blockp", name="blockp_t")
            bpost_t = consts.tile([128, 128], BF16, tag="bpost", name="bpost_t")

            def emit_big_consts():
                nc.sync.dma_start(wrep_t[:], wrep_d[:])
                nc.sync.dma_start(blockp_t[:], blockp_d[:])
                nc.sync.dma_start(bpost_t[:], bpost_d[:])

            state = {}

            def emit_load(th, pieces=1):
                xt = x_pool.tile([128, 2, C], BF16, tag="xsup",
                                 name=f"x_{th}")
                w = C // pieces
                for j in range(pieces):
                    nc.sync.dma_start(
                        out=xt[:, :, j * w : (j + 1) * w],
                        in_=xv[th][:, :, j * w : (j + 1) * w],
                    )
                t, h = divmod(th, 2)
                if t not in state:
                    state[t] = [[None, None], None, None]
                state[t][0][h] = xt

            def emit_agg(t):
                """x_agg via 4 concurrent col-tiled matmuls per 512-chunk.

                PSUM partition 32s+bg holds x_agg of supertile row 32s+bg.
                DVE evacuates fused with the rmsnorm-weight multiply; ACT
                squares the (unrounded) PSUM values with accum_out.
                """
                xh = state[t][0]
                xaggw = aggw_pool.tile([128, C], BF16, tag="xaggw",
                                       name=f"xaggw_{t}")
                sq8 = small_pool.tile([128, NCH], F32, tag="sq8",
                                      name=f"sq8_{t}")
                ats = {}
                # Emit the half-A col-tiles for all chunks first: they only
                # depend on the first 2MB load, so the PE can start while
                # half B is still in flight (PE queue is FIFO).
                for q in range(NCH):
                    at = agg_pool.tile([128, CH], F32, tag="agg",
                                       name=f"agg_{t}_{q}")
                    ats[q] = at
                    for s in (0, 1):
                        nc.tensor.matmul(
                            at[32 * s : 32 * s + 32, :],
                            wpre_t[:],
                            xh[0][:, s, CH * q : CH * (q + 1)],
                            start=True,
                            stop=True,
                            tile_position=(0, 32 * s),
                        )
                    if q == NCH // 2 - 1:
                        break
                for q in range(NCH):
                    if q < NCH // 2:
                        at = ats[q]
                    else:
                        at = agg_pool.tile([128, CH], F32, tag="agg",
                                           name=f"agg_{t}_{q}")
                        for s in (0, 1):
                            nc.tensor.matmul(
                                at[32 * s : 32 * s + 32, :],
                                wpre_t[:],
                                xh[0][:, s, CH * q : CH * (q + 1)],
                                start=True,
                                stop=True,
                                tile_position=(0, 32 * s),
                            )
                    for s in (2, 3):
                        nc.tensor.matmul(
                            at[32 * s : 32 * s + 32, :],
                            wpre_t[:],
                            xh[1][:, s - 2, CH * q : CH * (q + 1)],
                            start=True,
                            stop=True,
                            tile_position=(0, 32 * s),
                        )
                    lo = CH * q
                    nc.vector.tensor_mul(
                        xaggw[:, lo : lo + CH], at[:], wrep_t[:, lo : lo + CH]
                    )
                    scr = scr_pool.tile([128, CH], BF16, tag="scr",
                                        name=f"scr_{t}_{q}")
                    nc.scalar.activation(
                        scr[:], at[:], Act.Square, accum_out=sq8[:, q : q + 1]
                    )
                state[t][1] = (xaggw, sq8)

            def emit_norm(t):
                xaggw, sq8 = state[t][1]
                sumsq = small_pool.tile([128, 1], F32, tag="sumsq",
                                        name=f"ss_{t}")
                nc.vector.tensor_reduce(
                    sumsq[:], sq8[:], mybir.AxisListType.X, AluOp.add
                )
                rmsv = small_pool.tile([128, 1], F32, tag="rmsv",
                                       name=f"rms_{t}")
                nc.scalar.activation(
                    rmsv[:], sumsq[:], Act.Sqrt, bias=eps_t[:], scale=1.0 / C
                )
                invr = small_pool.tile([128, 1], F32, tag="invr",
                                       name=f"invr_{t}")
                nc.vector.reciprocal(invr[:], rmsv[:])
                yn = yn_pool.tile([128, C], BF16, tag="yn", name=f"yn_{t}")
                nc.vector.tensor_scalar_mul(yn[:], xaggw[:], invr[:])
                state[t][2] = yn

            def emit_mix(t, split_store=False):
                """mix + post per 512-chunk, two subtiles at a time.

                blockp is a full-array matmul; the two posts are K=32
                row-tiles at tile_position (32s, 0) and run concurrently.
                Evacuation alternates DVE / ACT.
                """
                xh, _, yn = state.pop(t)
                evac = 0
                for pair in ((0, 1), (2, 3)):
                    h = pair[0] // 2
                    osb = out_pool.tile([128, 2, C], BF16, tag="osb",
                                        name=f"osb_{t}_{h}")
                    for q in range(NCH):
                        mts = []
                        for s in pair:
                            mt = mix_pool.tile([128, CH], F32, tag=f"mix{s % 2}",
                                           name=f"mix_{t}_{s}_{q}")
                            nc.tensor.matmul(
                                mt[:],
                                blockp_t[:],
                                xh[s // 2][:, s % 2, CH * q : CH * (q + 1)],
                                start=True,
                                stop=False,
                            )
                            mts.append((s, mt))
                        for s, mt in mts:
                            nc.tensor.matmul(
                                mt[:],
                                bpost_t[32 * s : 32 * s + 32, :],
                                yn[32 * s : 32 * s + 32, CH * q : CH * (q + 1)],
                                start=False,
                                stop=True,
                                tile_position=(32 * s, 0),
                            )
                        for s, mt in mts:
                            dst = osb[:, s % 2, CH * q : CH * (q + 1)]
                            if evac % 2 == 0:
                                nc.vector.tensor_copy(dst, mt[:])
                            else:
                                nc.scalar.copy(dst, mt[:])
                            evac += 1
                    nc.gpsimd.dma_start(out=ov[2 * t + h], in_=osb[:])

            n_th = 2 * n_st
            emit_load(0, pieces=4)
            emit_load(1, pieces=2)
            emit_big_consts()
            for th in (2, 3):
                if th < n_th:
                    emit_load(th)
            for t in range(n_st):
                for th in (2 * t + 4, 2 * t + 5):
                    if th < n_th:
                        emit_load(th)
                if t == 0:
                    emit_agg(0)
                emit_norm(t)
                if t + 1 < n_st:
                    emit_agg(t + 1)
                emit_mix(t, split_store=(t == n_st - 1))

    nc.compile()
    return nc


def _sigmoid_f32(x):
    x = np.asarray(x, np.float32)
    return (1.0 / (1.0 + np.exp(-x.astype(np.float64)))).astype(np.float32)


def _host_matrices(rmsnorm_weight, H_pre, H_post, H_res):
    f32 = np.float32
    hpre = _sigmoid_f32(H_pre)                        # [4]
    hpost = (2.0 * _sigmoid_f32(H_post)).astype(f32)  # [4]
    P = np.exp(np.asarray(H_res, f32))
    for _ in range(SINKHORN_ITERS):
        P = P / (P.sum(axis=-1, keepdims=True) + f32(EPS))
        P = P / (P.sum(axis=-2, keepdims=True) + f32(EPS))
    P = P.astype(f32)

    # mix: out[(bg,i), c] = sum_j blockp[(bg,j), (bg,i)] * x[(bg,j), c]
    blockp = np.zeros((128, 128), f32)
    for bg in range(SUB):
        blockp[4 * bg : 4 * bg + 4, 4 * bg : 4 * bg + 4] = P.T

    # agg col-tile weights (same for every col group s):
    #   out[col 32s+bg2, c] = sum_(bg,n) wpre[(bg,n), bg2] * x_s[(bg,n), c]
    wpre = np.zeros((128, SUB), f32)
    for bg in range(SUB):
        for n in range(4):
            wpre[4 * bg + n, bg] = hpre[n]

    # post row-tile weights: rows 32s.. hold the K=32 lhsT for subtile s:
    #   out[(bg,i), c] += sum_bg2 bpost[32s+bg2, (bg,i)] * yn[32s+bg2, c]
    bpost = np.zeros((128, 128), f32)
    for s in range(NSUB):
        for bg in range(SUB):
            for i in range(4):
                bpost[32 * s + bg, 4 * bg + i] = hpost[i]

    wrep = np.broadcast_to(
        np.asarray(rmsnorm_weight, f32)[None, :], (128, C)
    )
    return {
        "wrep": np.ascontiguousarray(wrep.astype(BF16_NP)),
        "blockp": blockp.astype(BF16_NP),
        "wpre": wpre.astype(BF16_NP),
        "bpost": bpost.astype(BF16_NP),
    }


def kernel(x, rmsnorm_weight, H_pre, H_post, H_res):
    global _PROGRAM, LAST_RESULTS
    x = np.asarray(x, np.float32)
    assert x.shape == (B, N, C), x.shape
    xbf = np.ascontiguousarray(x.astype(BF16_NP))

    if _PROGRAM is None:
        _PROGRAM = _build_program()
    nc = _PROGRAM

    consts = _host_matrices(rmsnorm_weight, H_pre, H_post, H_res)
    shards = np.split(xbf, NCORES, axis=0)
    in_maps = [{"x": np.ascontiguousarray(s), **consts} for s in shards]

    trace = bool(int(os.environ.get("MHC_TRACE", "0")))
    br = run_bass_kernel_spmd(
        nc, in_maps, core_ids=list(range(NCORES)), trace=trace
    )
    LAST_RESULTS = br
    out = np.concatenate([r["out"] for r in br.results], axis=0)
    return out.astype(np.float32)


# revision 25
# speedup vs baseline: 1.1098x; 1.0631x over previous
"""Trainium2 Bass kernel for the MHC layer (nn_MHCLayer_20555713478899).

Reference computation (per batch row b of x[B=8192, n=4, C=4096] f32):
    hpre = sigmoid(H_pre)                     # [4]
    x_agg[b, c]   = sum_n hpre[n] * x[b, n, c]
    x_agg_bf      = bf16_roundtrip(x_agg)
    rms[b]        = sqrt(mean_c(x_agg_bf^2) + 1e-6)
    y_norm[b, c]  = x_agg_bf / rms * rmsnorm_weight[c]
    P             = sinkhorn3(exp(H_res))     # [4, 4]  (tiny, host-computed)
    hpost = 2*sigmoid(H_post)                 # [4]
    out[b, i, c]  = sum_j P[i, j] * x[b, j, c] + hpost[i] * y_norm[b, c]

Strategy: data-parallel shard of B across 8 NeuronCores (1024 rows each).
On-chip, batches are processed in supertiles of 128 rows = 4 subtiles of 32
rows.  A supertile loads as ONE 4MB DMA into a [128, 4*4096] SBUF tile whose
partition index is (bg*4 + n) and whose free index is (s*4096 + c); the
n-mixing then runs as 128-partition matmuls with small block matrices:
  agg : 4 concurrent col-tiles (M=32, tile_position=(0,32s)) write
        x_agg rows 32s+bg of one [128,512] PSUM chunk
  mix : full-array blockP (block-diagonal P.T), one mm per subtile chunk
  post: row-tiles (K=32, tile_position=(32s,0)) accumulate
        hpost[i]*y_norm into the mix PSUM, 2 subtiles concurrently
The RMS-norm path: ACT squares the agg PSUM directly (accum_out), DVE
evacuates agg PSUM fused with the rmsnorm weight multiply, Rsqrt gives
1/rms, and y_norm = xagg_w * invr on DVE.  Mix PSUM is evacuated to bf16
by alternating DVE/ACT copies.

HBM traffic is halved vs the f32 version: x is cast to bf16 on the host
(the device consumed bf16 for every matmul anyway) and the output is
stored as bf16 and upcast to f32 on the host.  Loads ride HWDGE (sync/SP
ring); stores ride SWDGE (gpsimd/Pool ring) so the two streams sit on
different DMA queues and interleave at packet granularity.
"""

import contextlib
import os

import numpy as np
import ml_dtypes

import concourse.bass as bass
import concourse.tile as tile
from concourse import bacc, mybir
from concourse.bass_utils import run_bass_kernel_spmd

B, N, C = 8192, 4, 4096
NCORES = 8
BLOC = B // NCORES          # 1024 batch rows per core
SUB = 32                    # batch rows per subtile (SUB*N = 128 partitions)
NSUB = 4                    # subtiles per supertile
ST = SUB * NSUB             # 128 batch rows per supertile
CH = 512                    # matmul / PSUM chunk width (one PSUM bank)
NCH = C // CH               # 8 chunks per subtile row
EPS = 1e-6
SINKHORN_ITERS = 3

F32 = mybir.dt.float32
BF16 = mybir.dt.bfloat16
BF16_NP = ml_dtypes.bfloat16

_PROGRAM = None
LAST_RESULTS = None         # BassKernelResults of the last run (for profiling)


def _build_program(bloc=BLOC):
    nc = bacc.Bacc("TRN2", target_bir_lowering=False)

    x_d = nc.dram_tensor("x", [bloc, N, C], BF16, kind="ExternalInput")
    wrep_d = nc.dram_tensor("wrep", [128, C], BF16, kind="ExternalInput")
    blockp_d = nc.dram_tensor("blockp", [128, 128], BF16, kind="ExternalInput")
    wpre_d = nc.dram_tensor("wpre", [128, SUB], BF16, kind="ExternalInput")
    bpost_d = nc.dram_tensor("bpost", [128, 128], BF16, kind="ExternalInput")
    out_d = nc.dram_tensor("out", [bloc, N, C], BF16, kind="ExternalOutput")

    n_st = bloc // ST
    AluOp = mybir.AluOpType
    Act = mybir.ActivationFunctionType

    # Half-supertile views: 64 rows = 2 subtiles; partition (bg n), free
    # (s c).  One 2MB DMA per half for finer load/store pipelining.
    xv = x_d[:].rearrange("(th s bg) n c -> th (bg n) s c", s=2, bg=SUB)
    ov = out_d[:].rearrange("(th s bg) n c -> th (bg n) s c", s=2, bg=SUB)

    with tile.TileContext(nc) as tc:
        with (
            tc.tile_pool(name="consts", bufs=1) as consts,
            tc.tile_pool(name="xsup", bufs=7) as x_pool,
            tc.tile_pool(name="aggw", bufs=2) as aggw_pool,
            tc.tile_pool(name="yn", bufs=2) as yn_pool,
            tc.tile_pool(name="scr", bufs=2) as scr_pool,
            tc.tile_pool(name="small", bufs=4) as small_pool,
            tc.tile_pool(name="osb", bufs=3) as out_pool,
            tc.tile_pool(name="aggps", bufs=4, space=bass.MemorySpace.PSUM) as agg_pool,
            tc.tile_pool(name="mixps", bufs=2, space=bass.MemorySpace.PSUM) as mix_pool,
        ):
            # wpre (8KB) is all the first agg matmuls need; the larger
            # consts are deferred until after the first two x loads so the
            # PE can start ~10us earlier.
            wpre_t = consts.tile([128, SUB], BF16, tag="wpre", name="wpre_t")
            nc.sync.dma_start(wpre_t[:], wpre_d[:])
            eps_t = consts.tile([128, 1], F32, tag="eps", name="eps_t")
            nc.vector.memset(eps_t[:], EPS)
            wrep_t = consts.tile([128, C], BF16, tag="wrep", name="wrep_t")
            blockp_t = consts.tile([128, 128], BF16, tag="blockp", name="blockp_t")
            bpost_t = consts.tile([128, 128], BF16, tag="bpost", name="bpost_t")

            def emit_big_consts():
                nc.sync.dma_start(blockp_t[:], blockp_d[:])
                nc.sync.dma_start(bpost_t[:], bpost_d[:])

            state = {}

            def emit_load(th, pieces=1):
                xt = x_pool.tile([128, 2, C], BF16, tag="xsup",
                                 name=f"x_{th}")
                w = C // pieces
                for j in range(pieces):
                    nc.sync.dma_start(
                        out=xt[:, :, j * w : (j + 1) * w],
                        in_=xv[th][:, :, j * w : (j + 1) * w],
                    )
                t, h = divmod(th, 2)
                if t not in state:
                    state[t] = [[None, None], None, None]
                state[t][0][h] = xt

            def emit_agg(t):
                """x_agg via 4 concurrent col-tiled matmuls per 512-chunk.

                PSUM partition 32s+bg holds x_agg of supertile row 32s+bg.
                DVE evacuates fused with the rmsnorm-weight multiply; ACT
                squares the (unrounded) PSUM values with accum_out.
                """
                xh = state[t][0]
                xaggw = aggw_pool.tile([128, C], BF16, tag="xaggw",
                                       name=f"xaggw_{t}")
                sq8 = small_pool.tile([128, NCH], F32, tag="sq8",
                                      name=f"sq8_{t}")
                ats = {}
                # Emit the half-A col-tiles for all chunks first: they only
                # depend on the first 2MB load, so the PE can start while
                # half B is still in flight (PE queue is FIFO).
                for q in range(NCH):
                    at = agg_pool.tile([128, CH], F32, tag="agg",
                                       name=f"agg_{t}_{q}")
                    ats[q] = at
                    for s in (0, 1):
                        nc.tensor.matmul(
                            at[32 * s : 32 * s + 32, :],
                            wpre_t[:],
                            xh[0][:, s, CH * q : CH * (q + 1)],
                            start=True,
                            stop=True,
                            tile_position=(0, 32 * s),
                        )
                    if q == NCH // 2 - 1:
                        break
                for q in range(NCH):
                    if q < NCH // 2:
                        at = ats[q]
                    else:
                        at = agg_pool.tile([128, CH], F32, tag="agg",
                                           name=f"agg_{t}_{q}")
                        for s in (0, 1):
                            nc.tensor.matmul(
                                at[32 * s : 32 * s + 32, :],
                                wpre_t[:],
                                xh[0][:, s, CH * q : CH * (q + 1)],
                                start=True,
                                stop=True,
                                tile_position=(0, 32 * s),
                            )
                    for s in (2, 3):
                        nc.tensor.matmul(
                            at[32 * s : 32 * s + 32, :],
                            wpre_t[:],
                            xh[1][:, s - 2, CH * q : CH * (q + 1)],
                            start=True,
                            stop=True,
                            tile_position=(0, 32 * s),
                        )
                    lo = CH * q
                    nc.vector.tensor_mul(
                        xaggw[:, lo : lo + CH], at[:], wrep_t[:, lo : lo + CH]
                    )
                    scr = scr_pool.tile([128, CH], BF16, tag="scr",
                                        name=f"scr_{t}_{q}")
                    nc.scalar.activation(
                        scr[:], at[:], Act.Square, accum_out=sq8[:, q : q + 1]
                    )
                state[t][1] = (xaggw, sq8)

            def emit_norm(t):
                xaggw, sq8 = state[t][1]
                sumsq = small_pool.tile([128, 1], F32, tag="sumsq",
                                        name=f"ss_{t}")
                nc.vector.tensor_reduce(
                    sumsq[:], sq8[:], mybir.AxisListType.X, AluOp.add
                )
                rmsv = small_pool.tile([128, 1], F32, tag="rmsv",
                                       name=f"rms_{t}")
                nc.scalar.activation(
                    rmsv[:], sumsq[:], Act.Sqrt, bias=eps_t[:], scale=1.0 / C
                )
                invr = small_pool.tile([128, 1], F32, tag="invr",
                                       name=f"invr_{t}")
                nc.vector.reciprocal(invr[:], rmsv[:])
                yn = yn_pool.tile([128, C], BF16, tag="yn", name=f"yn_{t}")
                nc.vector.tensor_scalar_mul(yn[:], xaggw[:], invr[:])
                state[t][2] = yn

            def emit_mix(t, split_store=False):
                """mix + post per 512-chunk, two subtiles at a time.

                blockp is a full-array matmul; the two posts are K=32
                row-tiles at tile_position (32s, 0) and run concurrently.
                Evacuation alternates DVE / ACT.
                """
                xh, _, yn = state.pop(t)
                evac = 0
                for pair in ((0, 1), (2, 3)):
                    h = pair[0] // 2
                    osb = out_pool.tile([128, 2, C], BF16, tag="osb",
                                        name=f"osb_{t}_{h}")
                    for q in range(NCH):
                        mts = []
                        for s in pair:
                            mt = mix_pool.tile([128, CH], F32, tag=f"mix{s % 2}",
                                           name=f"mix_{t}_{s}_{q}")
                            nc.tensor.matmul(
                                mt[:],
                                blockp_t[:],
                                xh[s // 2][:, s % 2, CH * q : CH * (q + 1)],
                                start=True,
                                stop=False,
                            )
                            mts.append((s, mt))
                        for s, mt in mts:
                            nc.tensor.matmul(
                                mt[:],
                                bpost_t[32 * s : 32 * s + 32, :],
                                yn[32 * s : 32 * s + 32, CH * q : CH * (q + 1)],
                                start=False,
                                stop=True,
                                tile_position=(32 * s, 0),
                            )
                        for s, mt in mts:
                            dst = osb[:, s % 2, CH * q : CH * (q + 1)]
                            if evac % 2 == 0:
                                nc.vector.tensor_copy(dst, mt[:])
                            else:
                                nc.scalar.copy(dst, mt[:])
                            evac += 1
                    nc.gpsimd.dma_start(out=ov[2 * t + h], in_=osb[:])

            n_th = 2 * n_st
            emit_load(0, pieces=4)
            nc.sync.dma_start(wrep_t[:], wrep_d[:])
            emit_load(1, pieces=2)
            emit_big_consts()
            for th in (2, 3, 4):
                if th < n_th:
                    emit_load(th)
            for t in range(n_st):
                for th in (2 * t + 5, 2 * t + 6):
                    if th < n_th:
                        emit_load(th)
                if t == 0:
                    emit_agg(0)
                emit_norm(t)
                if t + 1 < n_st:
                    emit_agg(t + 1)
                emit_mix(t, split_store=(t == n_st - 1))

    nc.compile()
    return nc


def _sigmoid_f32(x):
    x = np.asarray(x, np.float32)
    return (1.0 / (1.0 + np.exp(-x.astype(np.float64)))).astype(np.float32)


def _host_matrices(rmsnorm_weight, H_pre, H_post, H_res):
    f32 = np.float32
    hpre = _sigmoid_f32(H_pre)                        # [4]
    hpost = (2.0 * _sigmoid_f32(H_post)).astype(f32)  # [4]
    P = np.exp(np.asarray(H_res, f32))
    for _ in range(SINKHORN_ITERS):
        P = P / (P.sum(axis=-1, keepdims=True) + f32(EPS))
        P = P / (P.sum(axis=-2, keepdims=True) + f32(EPS))
    P = P.astype(f32)

    # mix: out[(bg,i), c] = sum_j blockp[(bg,j), (bg,i)] * x[(bg,j), c]
    blockp = np.zeros((128, 128), f32)
    for bg in range(SUB):
        blockp[4 * bg : 4 * bg + 4, 4 * bg : 4 * bg + 4] = P.T

    # agg col-tile weights (same for every col group s):
    #   out[col 32s+bg2, c] = sum_(bg,n) wpre[(bg,n), bg2] * x_s[(bg,n), c]
    wpre = np.zeros((128, SUB), f32)
    for bg in range(SUB):
        for n in range(4):
            wpre[4 * bg + n, bg] = hpre[n]

    # post row-tile weights: rows 32s.. hold the K=32 lhsT for subtile s:
    #   out[(bg,i), c] += sum_bg2 bpost[32s+bg2, (bg,i)] * yn[32s+bg2, c]
    bpost = np.zeros((128, 128), f32)
    for s in range(NSUB):
        for bg in range(SUB):
            for i in range(4):
                bpost[32 * s + bg, 4 * bg + i] = hpost[i]

    wrep = np.broadcast_to(
        np.asarray(rmsnorm_weight, f32)[None, :], (128, C)
    )
    return {
        "wrep": np.ascontiguousarray(wrep.astype(BF16_NP)),
        "blockp": blockp.astype(BF16_NP),
        "wpre": wpre.astype(BF16_NP),
        "bpost": bpost.astype(BF16_NP),
    }


def kernel(x, rmsnorm_weight, H_pre, H_post, H_res):
    global _PROGRAM, LAST_RESULTS
    x = np.asarray(x, np.float32)
    assert x.shape == (B, N, C), x.shape
    xbf = np.ascontiguousarray(x.astype(BF16_NP))

    if _PROGRAM is None:
        _PROGRAM = _build_program()
    nc = _PROGRAM

    consts = _host_matrices(rmsnorm_weight, H_pre, H_post, H_res)
    shards = np.split(xbf, NCORES, axis=0)
    in_maps = [{"x": np.ascontiguousarray(s), **consts} for s in shards]

    trace = bool(int(os.environ.get("MHC_TRACE", "0")))
    br = run_bass_kernel_spmd(
        nc, in_maps, core_ids=list(range(NCORES)), trace=trace
    )
    LAST_RESULTS = br
    out = np.concatenate([r["out"] for r in br.results], axis=0)
    return out.astype(np.float32)


# revision 26
# speedup vs baseline: 1.1166x; 1.0062x over previous
"""Trainium2 Bass kernel for the MHC layer (nn_MHCLayer_20555713478899).

Reference computation (per batch row b of x[B=8192, n=4, C=4096] f32):
    hpre = sigmoid(H_pre)                     # [4]
    x_agg[b, c]   = sum_n hpre[n] * x[b, n, c]
    x_agg_bf      = bf16_roundtrip(x_agg)
    rms[b]        = sqrt(mean_c(x_agg_bf^2) + 1e-6)
    y_norm[b, c]  = x_agg_bf / rms * rmsnorm_weight[c]
    P             = sinkhorn3(exp(H_res))     # [4, 4]  (tiny, host-computed)
    hpost = 2*sigmoid(H_post)                 # [4]
    out[b, i, c]  = sum_j P[i, j] * x[b, j, c] + hpost[i] * y_norm[b, c]

Strategy: data-parallel shard of B across 8 NeuronCores (1024 rows each).
On-chip, batches are processed in supertiles of 128 rows = 4 subtiles of 32
rows.  A supertile loads as ONE 4MB DMA into a [128, 4*4096] SBUF tile whose
partition index is (bg*4 + n) and whose free index is (s*4096 + c); the
n-mixing then runs as 128-partition matmuls with small block matrices:
  agg : 4 concurrent col-tiles (M=32, tile_position=(0,32s)) write
        x_agg rows 32s+bg of one [128,512] PSUM chunk
  mix : full-array blockP (block-diagonal P.T), one mm per subtile chunk
  post: row-tiles (K=32, tile_position=(32s,0)) accumulate
        hpost[i]*y_norm into the mix PSUM, 2 subtiles concurrently
The RMS-norm path: ACT squares the agg PSUM directly (accum_out), DVE
evacuates agg PSUM fused with the rmsnorm weight multiply, Rsqrt gives
1/rms, and y_norm = xagg_w * invr on DVE.  Mix PSUM is evacuated to bf16
by alternating DVE/ACT copies.

HBM traffic is halved vs the f32 version: x is cast to bf16 on the host
(the device consumed bf16 for every matmul anyway) and the output is
stored as bf16 and upcast to f32 on the host.  Loads ride HWDGE (sync/SP
ring); stores ride SWDGE (gpsimd/Pool ring) so the two streams sit on
different DMA queues and interleave at packet granularity.
"""

import contextlib
import os

import numpy as np
import ml_dtypes

import concourse.bass as bass
import concourse.tile as tile
from concourse import bacc, mybir
from concourse.bass_utils import run_bass_kernel_spmd

B, N, C = 8192, 4, 4096
NCORES = 8
BLOC = B // NCORES          # 1024 batch rows per core
SUB = 32                    # batch rows per subtile (SUB*N = 128 partitions)
NSUB = 4                    # subtiles per supertile
ST = SUB * NSUB             # 128 batch rows per supertile
CH = 512                    # matmul / PSUM chunk width (one PSUM bank)
NCH = C // CH               # 8 chunks per subtile row
EPS = 1e-6
SINKHORN_ITERS = 3

F32 = mybir.dt.float32
BF16 = mybir.dt.bfloat16
BF16_NP = ml_dtypes.bfloat16

_PROGRAM = None
LAST_RESULTS = None         # BassKernelResults of the last run (for profiling)


def _build_program(bloc=BLOC):
    nc = bacc.Bacc("TRN2", target_bir_lowering=False)

    x_d = nc.dram_tensor("x", [bloc, N, C], BF16, kind="ExternalInput")
    wrep_d = nc.dram_tensor("wrep", [128, C], BF16, kind="ExternalInput")
    blockp_d = nc.dram_tensor("blockp", [128, 128], BF16, kind="ExternalInput")
    wpre_d = nc.dram_tensor("wpre", [128, SUB], BF16, kind="ExternalInput")
    bpost_d = nc.dram_tensor("bpost", [128, 128], BF16, kind="ExternalInput")
    out_d = nc.dram_tensor("out", [bloc, N, C], BF16, kind="ExternalOutput")

    n_st = bloc // ST
    AluOp = mybir.AluOpType
    Act = mybir.ActivationFunctionType

    # Half-supertile views: 64 rows = 2 subtiles; partition (bg n), free
    # (s c).  One 2MB DMA per half for finer load/store pipelining.
    xv = x_d[:].rearrange("(th s bg) n c -> th (bg n) s c", s=2, bg=SUB)
    ov = out_d[:].rearrange("(th s bg) n c -> th (bg n) s c", s=2, bg=SUB)

    with tile.TileContext(nc) as tc:
        with (
            tc.tile_pool(name="consts", bufs=1) as consts,
            tc.tile_pool(name="xsup", bufs=7) as x_pool,
            tc.tile_pool(name="aggw", bufs=2) as aggw_pool,
            tc.tile_pool(name="yn", bufs=2) as yn_pool,
            tc.tile_pool(name="scr", bufs=2) as scr_pool,
            tc.tile_pool(name="small", bufs=4) as small_pool,
            tc.tile_pool(name="osb", bufs=3) as out_pool,
            tc.tile_pool(name="aggps", bufs=4, space=bass.MemorySpace.PSUM) as agg_pool,
            tc.tile_pool(name="mixps", bufs=2, space=bass.MemorySpace.PSUM) as mix_pool,
        ):
            # wpre (8KB) is all the first agg matmuls need; the larger
            # consts are deferred until after the first two x loads so the
            # PE can start ~10us earlier.
            wpre_t = consts.tile([128, SUB], BF16, tag="wpre", name="wpre_t")
            nc.sync.dma_start(wpre_t[:], wpre_d[:])
            eps_t = consts.tile([128, 1], F32, tag="eps", name="eps_t")
            nc.vector.memset(eps_t[:], EPS)
            # Touch Square and Sqrt once now so their ACT table loads
            # (~1.3us each + drain) overlap the initial DMAs instead of
            # gating the first supertile's rms chain.
            warm_t = consts.tile([128, 1], F32, tag="actwarm", name="warm_t")
            nc.scalar.activation(warm_t[:], eps_t[:], Act.Square)
            nc.scalar.activation(warm_t[:], eps_t[:], Act.Sqrt)
            wrep_t = consts.tile([128, C], BF16, tag="wrep", name="wrep_t")
            blockp_t = consts.tile([128, 128], BF16, tag="blockp", name="blockp_t")
            bpost_t = consts.tile([128, 128], BF16, tag="bpost", name="bpost_t")

            def emit_big_consts():
                nc.sync.dma_start(blockp_t[:], blockp_d[:])
                nc.sync.dma_start(bpost_t[:], bpost_d[:])

            state = {}

            def emit_load(th, pieces=1):
                xt = x_pool.tile([128, 2, C], BF16, tag="xsup",
                                 name=f"x_{th}")
                w = C // pieces
                for j in range(pieces):
                    nc.sync.dma_start(
                        out=xt[:, :, j * w : (j + 1) * w],
                        in_=xv[th][:, :, j * w : (j + 1) * w],
                    )
                t, h = divmod(th, 2)
                if t not in state:
                    state[t] = [[None, None], None, None]
                state[t][0][h] = xt

            def emit_agg(t):
                """x_agg via 4 concurrent col-tiled matmuls per 512-chunk.

                PSUM partition 32s+bg holds x_agg of supertile row 32s+bg.
                DVE evacuates fused with the rmsnorm-weight multiply; ACT
                squares the (unrounded) PSUM values with accum_out.
                """
                xh = state[t][0]
                xaggw = aggw_pool.tile([128, C], BF16, tag="xaggw",
                                       name=f"xaggw_{t}")
                sq8 = small_pool.tile([128, NCH], F32, tag="sq8",
                                      name=f"sq8_{t}")
                ats = {}
                # Emit the half-A col-tiles for all chunks first: they only
                # depend on the first 2MB load, so the PE can start while
                # half B is still in flight (PE queue is FIFO).
                for q in range(NCH):
                    at = agg_pool.tile([128, CH], F32, tag="agg",
                                       name=f"agg_{t}_{q}")
                    ats[q] = at
                    for s in (0, 1):
                        nc.tensor.matmul(
                            at[32 * s : 32 * s + 32, :],
                            wpre_t[:],
                            xh[0][:, s, CH * q : CH * (q + 1)],
                            start=True,
                            stop=True,
                            tile_position=(0, 32 * s),
                        )
                    if q == NCH // 2 - 1:
                        break
                for q in range(NCH):
                    if q < NCH // 2:
                        at = ats[q]
                    else:
                        at = agg_pool.tile([128, CH], F32, tag="agg",
                                           name=f"agg_{t}_{q}")
                        for s in (0, 1):
                            nc.tensor.matmul(
                                at[32 * s : 32 * s + 32, :],
                                wpre_t[:],
                                xh[0][:, s, CH * q : CH * (q + 1)],
                                start=True,
                                stop=True,
                                tile_position=(0, 32 * s),
                            )
                    for s in (2, 3):
                        nc.tensor.matmul(
                            at[32 * s : 32 * s + 32, :],
                            wpre_t[:],
                            xh[1][:, s - 2, CH * q : CH * (q + 1)],
                            start=True,
                            stop=True,
                            tile_position=(0, 32 * s),
                        )
                    lo = CH * q
                    nc.vector.tensor_mul(
                        xaggw[:, lo : lo + CH], at[:], wrep_t[:, lo : lo + CH]
                    )
                    scr = scr_pool.tile([128, CH], BF16, tag="scr",
                                        name=f"scr_{t}_{q}")
                    nc.scalar.activation(
                        scr[:], at[:], Act.Square, accum_out=sq8[:, q : q + 1]
                    )
                state[t][1] = (xaggw, sq8)

            def emit_norm(t):
                xaggw, sq8 = state[t][1]
                sumsq = small_pool.tile([128, 1], F32, tag="sumsq",
                                        name=f"ss_{t}")
                nc.vector.tensor_reduce(
                    sumsq[:], sq8[:], mybir.AxisListType.X, AluOp.add
                )
                rmsv = small_pool.tile([128, 1], F32, tag="rmsv",
                                       name=f"rms_{t}")
                nc.scalar.activation(
                    rmsv[:], sumsq[:], Act.Sqrt, bias=eps_t[:], scale=1.0 / C
                )
                invr = small_pool.tile([128, 1], F32, tag="invr",
                                       name=f"invr_{t}")
                nc.vector.reciprocal(invr[:], rmsv[:])
                yn = yn_pool.tile([128, C], BF16, tag="yn", name=f"yn_{t}")
                nc.vector.tensor_scalar_mul(yn[:], xaggw[:], invr[:])
                state[t][2] = yn

            def emit_mix(t, split_store=False):
                """mix + post per 512-chunk, two subtiles at a time.

                blockp is a full-array matmul; the two posts are K=32
                row-tiles at tile_position (32s, 0) and run concurrently.
                Evacuation alternates DVE / ACT.
                """
                xh, _, yn = state.pop(t)
                evac = 0
                for pair in ((0, 1), (2, 3)):
                    h = pair[0] // 2
                    osb = out_pool.tile([128, 2, C], BF16, tag="osb",
                                        name=f"osb_{t}_{h}")
                    for q in range(NCH):
                        mts = []
                        for s in pair:
                            mt = mix_pool.tile([128, CH], F32, tag=f"mix{s % 2}",
                                           name=f"mix_{t}_{s}_{q}")
                            nc.tensor.matmul(
                                mt[:],
                                blockp_t[:],
                                xh[s // 2][:, s % 2, CH * q : CH * (q + 1)],
                                start=True,
                                stop=False,
                            )
                            mts.append((s, mt))
                        for s, mt in mts:
                            nc.tensor.matmul(
                                mt[:],
                                bpost_t[32 * s : 32 * s + 32, :],
                                yn[32 * s : 32 * s + 32, CH * q : CH * (q + 1)],
                                start=False,
                                stop=True,
                                tile_position=(32 * s, 0),
                            )
                        for s, mt in mts:
                            dst = osb[:, s % 2, CH * q : CH * (q + 1)]
                            if evac % 2 == 0:
                                nc.vector.tensor_copy(dst, mt[:])
                            else:
                                nc.scalar.copy(dst, mt[:])
                            evac += 1
                    nc.gpsimd.dma_start(out=ov[2 * t + h], in_=osb[:])

            n_th = 2 * n_st
            emit_load(0, pieces=4)
            nc.sync.dma_start(wrep_t[:], wrep_d[:])
            emit_load(1, pieces=2)
            emit_big_consts()
            for th in (2, 3, 4):
                if th < n_th:
                    emit_load(th)
            for t in range(n_st):
                for th in (2 * t + 5, 2 * t + 6):
                    if th < n_th:
                        emit_load(th)
                if t == 0:
                    emit_agg(0)
                emit_norm(t)
                if t + 1 < n_st:
                    emit_agg(t + 1)
                emit_mix(t, split_store=(t == n_st - 1))

    nc.compile()
    return nc


def _sigmoid_f32(x):
    x = np.asarray(x, np.float32)
    return (1.0 / (1.0 + np.exp(-x.astype(np.float64)))).astype(np.float32)


def _host_matrices(rmsnorm_weight, H_pre, H_post, H_res):
    f32 = np.float32
    hpre = _sigmoid_f32(H_pre)                        # [4]
    hpost = (2.0 * _sigmoid_f32(H_post)).astype(f32)  # [4]
    P = np.exp(np.asarray(H_res, f32))
    for _ in range(SINKHORN_ITERS):
        P = P / (P.sum(axis=-1, keepdims=True) + f32(EPS))
        P = P / (P.sum(axis=-2, keepdims=True) + f32(EPS))
    P = P.astype(f32)

    # mix: out[(bg,i), c] = sum_j blockp[(bg,j), (bg,i)] * x[(bg,j), c]
    blockp = np.zeros((128, 128), f32)
    for bg in range(SUB):
        blockp[4 * bg : 4 * bg + 4, 4 * bg : 4 * bg + 4] = P.T

    # agg col-tile weights (same for every col group s):
    #   out[col 32s+bg2, c] = sum_(bg,n) wpre[(bg,n), bg2] * x_s[(bg,n), c]
    wpre = np.zeros((128, SUB), f32)
    for bg in range(SUB):
        for n in range(4):
            wpre[4 * bg + n, bg] = hpre[n]

    # post row-tile weights: rows 32s.. hold the K=32 lhsT for subtile s:
    #   out[(bg,i), c] += sum_bg2 bpost[32s+bg2, (bg,i)] * yn[32s+bg2, c]
    bpost = np.zeros((128, 128), f32)
    for s in range(NSUB):
        for bg in range(SUB):
            for i in range(4):
                bpost[32 * s + bg, 4 * bg + i] = hpost[i]

    wrep = np.broadcast_to(
        np.asarray(rmsnorm_weight, f32)[None, :], (128, C)
    )
    return {
        "wrep": np.ascontiguousarray(wrep.astype(BF16_NP)),
        "blockp": blockp.astype(BF16_NP),
        "wpre": wpre.astype(BF16_NP),
        "bpost": bpost.astype(BF16_NP),
    }


def kernel(x, rmsnorm_weight, H_pre, H_post, H_res):
    global _PROGRAM, LAST_RESULTS
    x = np.asarray(x, np.float32)
    assert x.shape == (B, N, C), x.shape
    xbf = np.ascontiguousarray(x.astype(BF16_NP))

    if _PROGRAM is None:
        _PROGRAM = _build_program()
    nc = _PROGRAM

    consts = _host_matrices(rmsnorm_weight, H_pre, H_post, H_res)
    shards = np.split(xbf, NCORES, axis=0)
    in_maps = [{"x": np.ascontiguousarray(s), **consts} for s in shards]

    trace = bool(int(os.environ.get("MHC_TRACE", "0")))
    br = run_bass_kernel_spmd(
        nc, in_maps, core_ids=list(range(NCORES)), trace=trace
    )
    LAST_RESULTS = br
    out = np.concatenate([r["out"] for r in br.results], axis=0)
    return out.astype(np.float32)


# revision 27
# speedup vs baseline: 1.1808x; 1.0575x over previous
"""Trainium2 Bass kernel for the MHC layer (nn_MHCLayer_20555713478899).

Reference computation (per batch row b of x[B=8192, n=4, C=4096] f32):
    hpre = sigmoid(H_pre)                     # [4]
    x_agg[b, c]   = sum_n hpre[n] * x[b, n, c]
    x_agg_bf      = bf16_roundtrip(x_agg)
    rms[b]        = sqrt(mean_c(x_agg_bf^2) + 1e-6)
    y_norm[b, c]  = x_agg_bf / rms * rmsnorm_weight[c]
    P             = sinkhorn3(exp(H_res))     # [4, 4]  (tiny, host-computed)
    hpost = 2*sigmoid(H_post)                 # [4]
    out[b, i, c]  = sum_j P[i, j] * x[b, j, c] + hpost[i] * y_norm[b, c]

Strategy: data-parallel shard of B across 8 NeuronCores (1024 rows each).
On-chip, batches are processed in supertiles of 128 rows = 4 subtiles of 32
rows.  A supertile loads as ONE 4MB DMA into a [128, 4*4096] SBUF tile whose
partition index is (bg*4 + n) and whose free index is (s*4096 + c); the
n-mixing then runs as 128-partition matmuls with small block matrices:
  agg : 4 concurrent col-tiles (M=32, tile_position=(0,32s)) write
        x_agg rows 32s+bg of one [128,512] PSUM chunk
  mix : full-array blockP (block-diagonal P.T), one mm per subtile chunk
  post: row-tiles (K=32, tile_position=(32s,0)) accumulate
        hpost[i]*y_norm into the mix PSUM, 2 subtiles concurrently
The RMS-norm path: ACT squares the agg PSUM directly (accum_out), DVE
evacuates agg PSUM fused with the rmsnorm weight multiply, Rsqrt gives
1/rms, and y_norm = xagg_w * invr on DVE.  Mix PSUM is evacuated to bf16
by alternating DVE/ACT copies.

HBM traffic is halved vs the f32 version: x is cast to bf16 on the host
(the device consumed bf16 for every matmul anyway) and the output is
stored as bf16 and upcast to f32 on the host.  Loads ride HWDGE (sync/SP
ring); stores ride SWDGE (gpsimd/Pool ring) so the two streams sit on
different DMA queues and interleave at packet granularity.
"""

import contextlib
import os

import numpy as np
import ml_dtypes

import concourse.bass as bass
import concourse.tile as tile
from concourse import bacc, mybir
from concourse.bass_utils import run_bass_kernel_spmd

B, N, C = 8192, 4, 4096
NCORES = 8
BLOC = B // NCORES          # 1024 batch rows per core
SUB = 32                    # batch rows per subtile (SUB*N = 128 partitions)
NSUB = 4                    # subtiles per supertile
ST = SUB * NSUB             # 128 batch rows per supertile
CH = 512                    # matmul / PSUM chunk width (one PSUM bank)
NCH = C // CH               # 8 chunks per subtile row
EPS = 1e-6
SINKHORN_ITERS = 3

F32 = mybir.dt.float32
BF16 = mybir.dt.bfloat16
BF16_NP = ml_dtypes.bfloat16

_PROGRAM = None
LAST_RESULTS = None         # BassKernelResults of the last run (for profiling)


def _build_program(bloc=BLOC):
    nc = bacc.Bacc("TRN2", target_bir_lowering=False)

    x_d = nc.dram_tensor("x", [bloc, N, C], BF16, kind="ExternalInput")
    wrep_d = nc.dram_tensor("wrep", [128, C], BF16, kind="ExternalInput")
    blockp_d = nc.dram_tensor("blockp", [128, 128], BF16, kind="ExternalInput")
    wpre_d = nc.dram_tensor("wpre", [128, SUB], BF16, kind="ExternalInput")
    bpost_d = nc.dram_tensor("bpost", [128, 128], BF16, kind="ExternalInput")
    out_d = nc.dram_tensor("out", [bloc, N, C], BF16, kind="ExternalOutput")

    n_st = bloc // ST
    AluOp = mybir.AluOpType
    Act = mybir.ActivationFunctionType

    # Half-supertile views: 64 rows = 2 subtiles; partition (bg n), free
    # (s c).  One 2MB DMA per half for finer load/store pipelining.
    xv = x_d[:].rearrange("(th s bg) n c -> th (bg n) s c", s=2, bg=SUB)
    ov = out_d[:].rearrange("(th s bg) n c -> th (bg n) s c", s=2, bg=SUB)

    with tile.TileContext(nc) as tc:
        with (
            tc.tile_pool(name="consts", bufs=1) as consts,
            tc.tile_pool(name="xsup", bufs=7) as x_pool,
            tc.tile_pool(name="aggw", bufs=2) as aggw_pool,
            tc.tile_pool(name="yn", bufs=2) as yn_pool,
            tc.tile_pool(name="scr", bufs=2) as scr_pool,
            tc.tile_pool(name="small", bufs=4) as small_pool,
            tc.tile_pool(name="osb", bufs=3) as out_pool,
            tc.tile_pool(name="aggps", bufs=4, space=bass.MemorySpace.PSUM) as agg_pool,
            tc.tile_pool(name="mixps", bufs=2, space=bass.MemorySpace.PSUM) as mix_pool,
        ):
            # wpre (8KB) is all the first agg matmuls need; the larger
            # consts are deferred until after the first two x loads so the
            # PE can start ~10us earlier.
            wpre_t = consts.tile([128, SUB], BF16, tag="wpre", name="wpre_t")
            nc.sync.dma_start(wpre_t[:], wpre_d[:])
            eps_t = consts.tile([128, 1], F32, tag="eps", name="eps_t")
            nc.vector.memset(eps_t[:], EPS)
            # Touch Square and Sqrt once now so their ACT table loads
            # (~1.3us each + drain) overlap the initial DMAs instead of
            # gating the first supertile's rms chain.
            warm_t = consts.tile([128, 1], F32, tag="actwarm", name="warm_t")
            nc.scalar.activation(warm_t[:], eps_t[:], Act.Square)
            nc.scalar.activation(warm_t[:], eps_t[:], Act.Sqrt)
            wrep_t = consts.tile([128, C], BF16, tag="wrep", name="wrep_t")
            blockp_t = consts.tile([128, 128], BF16, tag="blockp", name="blockp_t")
            bpost_t = consts.tile([128, 128], BF16, tag="bpost", name="bpost_t")

            def emit_big_consts():
                nc.sync.dma_start(blockp_t[:], blockp_d[:])
                nc.sync.dma_start(bpost_t[:], bpost_d[:])

            state = {}

            def emit_load(th, pieces=1):
                xt = x_pool.tile([128, 2, C], BF16, tag="xsup",
                                 name=f"x_{th}")
                w = C // pieces
                for j in range(pieces):
                    nc.sync.dma_start(
                        out=xt[:, :, j * w : (j + 1) * w],
                        in_=xv[th][:, :, j * w : (j + 1) * w],
                    )
                t, h = divmod(th, 2)
                if t not in state:
                    state[t] = [[None, None], None, None]
                state[t][0][h] = xt

            def emit_agg(t):
                """x_agg via 4 concurrent col-tiled matmuls per 512-chunk.

                PSUM partition 32s+bg holds x_agg of supertile row 32s+bg.
                DVE evacuates fused with the rmsnorm-weight multiply; ACT
                squares the (unrounded) PSUM values with accum_out.
                """
                xh = state[t][0]
                xaggw = aggw_pool.tile([128, C], BF16, tag="xaggw",
                                       name=f"xaggw_{t}")
                sq8 = small_pool.tile([128, NCH], F32, tag="sq8",
                                      name=f"sq8_{t}")
                ats = {}
                # Emit the half-A col-tiles for all chunks first: they only
                # depend on the first 2MB load, so the PE can start while
                # half B is still in flight (PE queue is FIFO).
                for q in range(NCH):
                    at = agg_pool.tile([128, CH], F32, tag="agg",
                                       name=f"agg_{t}_{q}")
                    ats[q] = at
                    for s in (0, 1):
                        nc.tensor.matmul(
                            at[32 * s : 32 * s + 32, :],
                            wpre_t[:],
                            xh[0][:, s, CH * q : CH * (q + 1)],
                            start=True,
                            stop=True,
                            tile_position=(0, 32 * s),
                        )
                    if q == NCH // 2 - 1:
                        break
                for q in range(NCH):
                    if q < NCH // 2:
                        at = ats[q]
                    else:
                        at = agg_pool.tile([128, CH], F32, tag="agg",
                                           name=f"agg_{t}_{q}")
                        for s in (0, 1):
                            nc.tensor.matmul(
                                at[32 * s : 32 * s + 32, :],
                                wpre_t[:],
                                xh[0][:, s, CH * q : CH * (q + 1)],
                                start=True,
                                stop=True,
                                tile_position=(0, 32 * s),
                            )
                    for s in (2, 3):
                        nc.tensor.matmul(
                            at[32 * s : 32 * s + 32, :],
                            wpre_t[:],
                            xh[1][:, s - 2, CH * q : CH * (q + 1)],
                            start=True,
                            stop=True,
                            tile_position=(0, 32 * s),
                        )
                    lo = CH * q
                    nc.vector.tensor_mul(
                        xaggw[:, lo : lo + CH], at[:], wrep_t[:, lo : lo + CH]
                    )
                    scr = scr_pool.tile([128, CH], BF16, tag="scr",
                                        name=f"scr_{t}_{q}")
                    nc.scalar.activation(
                        scr[:], at[:], Act.Square, accum_out=sq8[:, q : q + 1]
                    )
                state[t][1] = (xaggw, sq8)

            def emit_norm(t):
                xaggw, sq8 = state[t][1]
                sumsq = small_pool.tile([128, 1], F32, tag="sumsq",
                                        name=f"ss_{t}")
                scr8 = small_pool.tile([128, NCH], F32, tag="scr8",
                                       name=f"scr8_{t}")
                # ACT-local reduce: squares are >=0 so Relu is identity and
                # accum_out sums them -- avoids a DVE round-trip in the
                # rms -> y_norm critical chain.
                nc.scalar.activation(
                    scr8[:], sq8[:], Act.Relu, accum_out=sumsq[:]
                )
                rmsv = small_pool.tile([128, 1], F32, tag="rmsv",
                                       name=f"rms_{t}")
                nc.scalar.activation(
                    rmsv[:], sumsq[:], Act.Sqrt, bias=eps_t[:], scale=1.0 / C
                )
                invr = small_pool.tile([128, 1], F32, tag="invr",
                                       name=f"invr_{t}")
                nc.vector.reciprocal(invr[:], rmsv[:])
                yn = yn_pool.tile([128, C], BF16, tag="yn", name=f"yn_{t}")
                nc.vector.tensor_scalar_mul(yn[:], xaggw[:], invr[:])
                state[t][2] = yn

            def emit_mix(t, split_store=False):
                """mix + post per 512-chunk, two subtiles at a time.

                blockp is a full-array matmul; the two posts are K=32
                row-tiles at tile_position (32s, 0) and run concurrently.
                Evacuation alternates DVE / ACT.
                """
                xh, _, yn = state.pop(t)
                evac = 0
                for pair in ((0, 1), (2, 3)):
                    h = pair[0] // 2
                    osb = out_pool.tile([128, 2, C], BF16, tag="osb",
                                        name=f"osb_{t}_{h}")
                    for q in range(NCH):
                        mts = []
                        for s in pair:
                            mt = mix_pool.tile([128, CH], F32, tag=f"mix{s % 2}",
                                           name=f"mix_{t}_{s}_{q}")
                            nc.tensor.matmul(
                                mt[:],
                                blockp_t[:],
                                xh[s // 2][:, s % 2, CH * q : CH * (q + 1)],
                                start=True,
                                stop=False,
                            )
                            mts.append((s, mt))
                        for s, mt in mts:
                            nc.tensor.matmul(
                                mt[:],
                                bpost_t[32 * s : 32 * s + 32, :],
                                yn[32 * s : 32 * s + 32, CH * q : CH * (q + 1)],
                                start=False,
                                stop=True,
                                tile_position=(32 * s, 0),
                            )
                        for s, mt in mts:
                            dst = osb[:, s % 2, CH * q : CH * (q + 1)]
                            if evac % 2 == 0:
                                nc.vector.tensor_copy(dst, mt[:])
                            else:
                                nc.scalar.copy(dst, mt[:])
                            evac += 1
                    nc.gpsimd.dma_start(out=ov[2 * t + h], in_=osb[:])

            n_th = 2 * n_st
            emit_load(0, pieces=4)
            nc.sync.dma_start(wrep_t[:], wrep_d[:])
            emit_load(1, pieces=2)
            emit_big_consts()
            for th in (2, 3, 4):
                if th < n_th:
                    emit_load(th)
            for t in range(n_st):
                for th in (2 * t + 5, 2 * t + 6):
                    if th < n_th:
                        emit_load(th)
                if t == 0:
                    emit_agg(0)
                emit_norm(t)
                if t + 1 < n_st:
                    emit_agg(t + 1)
                emit_mix(t, split_store=(t >= n_st - 2))

    nc.compile()
    return nc


def _sigmoid_f32(x):
    x = np.asarray(x, np.float32)
    return (1.0 / (1.0 + np.exp(-x.astype(np.float64)))).astype(np.float32)


def _host_matrices(rmsnorm_weight, H_pre, H_post, H_res):
    f32 = np.float32
    hpre = _sigmoid_f32(H_pre)                        # [4]
    hpost = (2.0 * _sigmoid_f32(H_post)).astype(f32)  # [4]
    P = np.exp(np.asarray(H_res, f32))
    for _ in range(SINKHORN_ITERS):
        P = P / (P.sum(axis=-1, keepdims=True) + f32(EPS))
        P = P / (P.sum(axis=-2, keepdims=True) + f32(EPS))
    P = P.astype(f32)

    # mix: out[(bg,i), c] = sum_j blockp[(bg,j), (bg,i)] * x[(bg,j), c]
    blockp = np.zeros((128, 128), f32)
    for bg in range(SUB):
        blockp[4 * bg : 4 * bg + 4, 4 * bg : 4 * bg + 4] = P.T

    # agg col-tile weights (same for every col group s):
    #   out[col 32s+bg2, c] = sum_(bg,n) wpre[(bg,n), bg2] * x_s[(bg,n), c]
    wpre = np.zeros((128, SUB), f32)
    for bg in range(SUB):
        for n in range(4):
            wpre[4 * bg + n, bg] = hpre[n]

    # post row-tile weights: rows 32s.. hold the K=32 lhsT for subtile s:
    #   out[(bg,i), c] += sum_bg2 bpost[32s+bg2, (bg,i)] * yn[32s+bg2, c]
    bpost = np.zeros((128, 128), f32)
    for s in range(NSUB):
        for bg in range(SUB):
            for i in range(4):
                bpost[32 * s + bg, 4 * bg + i] = hpost[i]

    wrep = np.broadcast_to(
        np.asarray(rmsnorm_weight, f32)[None, :], (128, C)
    )
    return {
        "wrep": np.ascontiguousarray(wrep.astype(BF16_NP)),
        "blockp": blockp.astype(BF16_NP),
        "wpre": wpre.astype(BF16_NP),
        "bpost": bpost.astype(BF16_NP),
    }


def kernel(x, rmsnorm_weight, H_pre, H_post, H_res):
    global _PROGRAM, LAST_RESULTS
    x = np.asarray(x, np.float32)
    assert x.shape == (B, N, C), x.shape
    xbf = np.ascontiguousarray(x.astype(BF16_NP))

    if _PROGRAM is None:
        _PROGRAM = _build_program()
    nc = _PROGRAM

    consts = _host_matrices(rmsnorm_weight, H_pre, H_post, H_res)
    shards = np.split(xbf, NCORES, axis=0)
    in_maps = [{"x": np.ascontiguousarray(s), **consts} for s in shards]

    trace = bool(int(os.environ.get("MHC_TRACE", "0")))
    br = run_bass_kernel_spmd(
        nc, in_maps, core_ids=list(range(NCORES)), trace=trace
    )
    LAST_RESULTS = br
    out = np.concatenate([r["out"] for r in br.results], axis=0)
    return out.astype(np.float32)
